# revision 4
# baseline (speedup 1.0000x reference)
"""INT4 MoE grouped-GEMM kernel for Trainium2 (8 NeuronCores).

Strategy
--------
The reference computes, per token t routed to expert e = expert_ids[t]:

    out[t, f] = sum_h inputs[t, h] * W[e, f, h],   W = (q - zp[e,f]) * scale[e,f]

where q is the int4-unpacked weight tensor. Because scale/zero_point are
per (expert, output-feature), the dequantization folds entirely into the
weights: we dequantize on the host into bf16 once, and the device kernel
is a pure grouped GEMM.

Sharding: output-feature parallel. Every core processes ALL tokens but
only a 1024-wide slice of the F=8192 output features of every expert.
This is perfectly load-balanced regardless of the token->expert
distribution (each core does exactly T*H*F/8 MACs) and needs no
collectives: core c produces out[:, 1024c:1024(c+1)].

Tokens are host-sorted by expert (they already arrive sorted; we argsort
anyway for robustness) and packed into 128-token M-tiles, zero-padded at
each expert boundary so every matmul is a uniform
[K=128, M=128] x [K=128, N=512] bf16 operation. The device loops:

  for e in experts:  load W_e^T slice [2048, 1024] (bf16, 4MB)
    for each 128-token tile of expert e:
      psum[128, 1024] = sum_{kt<16} xT_tile[kt].T @ WT_e[kt]   (32 matmuls)
      evict psum -> sbuf (DVE), DMA to out rows

Host gathers the 8 per-core [Tpad, 1024] outputs, drops pad rows, and
inverse-permutes back to original token order.
"""

import numpy as np
import ml_dtypes

E = 8          # experts
T = 8192       # tokens
H = 2048       # hidden (contraction)
F = 8192       # output features
NCORES = 8
FC = F // NCORES       # 1024 output features per core
KT = H // 128          # 16 k-tiles of 128
NB = FC // 512         # 2 PSUM banks per M-tile
BF16 = ml_dtypes.bfloat16

_PROGRAM_CACHE: dict[tuple, object] = {}
LAST_RESULT = None  # populated with BassKernelResults for external inspection


def _build_program(tiles_per_expert: tuple[int, ...]):
    """Build the SPMD Bass program. Structure depends only on the number of
    128-token tiles each expert owns (same program runs on all 8 cores)."""
    import concourse.bass as bass  # noqa: F401
    import concourse.mybir as mybir
    import concourse.tile as tile
    from concourse import bacc
    from concourse.bass import ts

    ntiles = int(sum(tiles_per_expert))
    nc = bacc.Bacc("TRN2", target_bir_lowering=False)
    xp = nc.declare_dram_parameter(
        "xp", [ntiles, 128, KT * 128], mybir.dt.bfloat16, isOutput=False
    )
    wT = nc.declare_dram_parameter(
        "wT", [E, H, FC], mybir.dt.bfloat16, isOutput=False
    )
    out = nc.declare_dram_parameter(
        "out", [ntiles * 128, FC], mybir.dt.float32, isOutput=True
    )

    # [E, H, FC] -> [E, 128(part), KT, FC] so one DMA loads a full expert slice
    wT_v = wT.rearrange("e (kt p) f -> e p kt f", p=128)

    with tile.TileContext(nc) as tc:
        with (
            tc.tile_pool(name="wpool", bufs=2) as wpool,
            tc.tile_pool(name="xpool", bufs=8) as xpool,
            tc.tile_pool(name="opool", bufs=6) as opool,
            tc.tile_pool(name="pspool", bufs=3, space="PSUM") as pspool,
        ):
            tile_idx = 0
            for e in range(E):
                if tiles_per_expert[e] == 0:
                    continue
                w_e = wpool.tile([128, KT, FC], mybir.dt.bfloat16, name="w_e")
                nc.sync.dma_start(out=w_e[:, :, :], in_=wT_v[e])
                for _ in range(tiles_per_expert[e]):
                    x_m = xpool.tile([128, KT, 128], mybir.dt.bfloat16, name="x_m")
                    nc.sync.dma_start(
                        out=x_m[:, :, :],
                        in_=xp[tile_idx].rearrange("p (kt t) -> p kt t", kt=KT),
                    )
                    ps = pspool.tile([128, FC], mybir.dt.float32, name="ps")
                    for kt in range(KT):
                        for nb in range(NB):
                            nc.tensor.matmul(
                                ps[:, ts(nb, 512)],
                                lhsT=x_m[:, kt, :],
                                rhs=w_e[:, kt, ts(nb, 512)],
                                start=(kt == 0),
                                stop=(kt == KT - 1),
                            )
                    o_m = opool.tile([128, FC], mybir.dt.float32, name="o_m")
                    nc.vector.tensor_copy(o_m[:, :], ps[:, :])
                    nc.sync.dma_start(
                        out=out[ts(tile_idx, 128), :], in_=o_m[:, :]
                    )
                    tile_idx += 1
    if not nc.is_finalized():
        nc.finalize()
    return nc


def kernel(
    packed_weights: np.ndarray,
    scales: np.ndarray,
    zero_points: np.ndarray,
    inputs: np.ndarray,
    expert_ids: np.ndarray,
    tokens_per_expert: np.ndarray,
    input_offsets: np.ndarray,
) -> np.ndarray:
    global LAST_RESULT
    from concourse.bass_utils import run_bass_kernel_spmd

    packed_weights = np.asarray(packed_weights)
    scales = np.asarray(scales, dtype=np.float32)
    zero_points = np.asarray(zero_points, dtype=np.float32)
    inputs = np.asarray(inputs, dtype=np.float32)
    expert_ids = np.asarray(expert_ids)

    # ---- host routing: sort tokens by expert (robust to unsorted input) ----
    perm = np.argsort(expert_ids, kind="stable")  # sorted order -> orig index
    counts = np.bincount(expert_ids, minlength=E).astype(np.int64)
    tiles_per_expert = tuple(int(-(-c // 128)) for c in counts)
    ntiles = int(sum(tiles_per_expert))

    # Row map: padded row r (of ntiles*128) -> original token index, -1 = pad
    row_map = np.full(ntiles * 128, -1, dtype=np.int64)
    pos = 0       # position in sorted token order
    row = 0       # padded row cursor
    for e in range(E):
        c = int(counts[e])
        nt = tiles_per_expert[e]
        if c:
            row_map[row : row + c] = perm[pos : pos + c]
        pos += c
        row += nt * 128

    # ---- host prep: x -> packed per-tile bf16 [ntiles, 128(k), KT*128(tok)] ----
    xs = np.ascontiguousarray(inputs.T)               # [H, T] fp32
    xs = np.concatenate([xs, np.zeros((H, 1), np.float32)], axis=1)  # pad col
    tok_idx = np.where(row_map >= 0, row_map, T)      # T -> zero column
    xg = xs[:, tok_idx].astype(BF16)                  # [H, ntiles*128]
    # (kt*128+p, i*128+t) -> (i, p, kt, t)
    xp_host = np.ascontiguousarray(
        xg.reshape(KT, 128, ntiles, 128).transpose(2, 1, 0, 3)
    ).reshape(ntiles, 128, KT * 128)

    # ---- host dequantize int4 weights -> bf16, transposed to [E, H, F] ----
    b = (packed_weights & 0xFF).astype(np.uint8)      # [E, F, P] byte values
    sc = scales[:, :, None]
    zp = zero_points[:, :, None]
    wlo = ((b & 0xF).astype(np.float32) - zp) * sc    # even h = 2p
    whi = ((b >> 4).astype(np.float32) - zp) * sc     # odd  h = 2p+1
    WT = np.empty((E, H, F), dtype=BF16)
    WT[:, 0::2, :] = wlo.transpose(0, 2, 1).astype(BF16)
    WT[:, 1::2, :] = whi.transpose(0, 2, 1).astype(BF16)

    # ---- build / fetch program ----
    key = tiles_per_expert
    nc = _PROGRAM_CACHE.get(key)
    if nc is None:
        nc = _build_program(tiles_per_expert)
        _PROGRAM_CACHE[key] = nc

    in_maps = []
    for c in range(NCORES):
        wT_c = np.ascontiguousarray(WT[:, :, c * FC : (c + 1) * FC])
        in_maps.append({"xp": xp_host, "wT": wT_c})

    res = run_bass_kernel_spmd(nc, in_maps, list(range(NCORES)))
    LAST_RESULT = res

    # ---- gather: concat feature slices, drop pad rows, unpermute ----
    out_sorted = np.concatenate(
        [res.results[c]["out"] for c in range(NCORES)], axis=1
    )  # [ntiles*128, F]
    valid = row_map >= 0
    out_full = np.empty((T, F), dtype=np.float32)
    out_full[row_map[valid]] = out_sorted[valid]
    return out_full
